# revision 26
# baseline (speedup 1.0000x reference)
"""GAT layer kernel for Trainium2, data-parallel over batch across 8 NeuronCores.

Per batch element b (one core each):
    hp  = h @ W_proj + b_proj                      # [N, D]
    s   = hp @ w_src ; t = hp @ w_dst              # [N]
    e   = relu(s[:,None] + t[None,:] + b_att)      # [N, N]
    att = exp(e) * a ; att /= att.sum(-1, keepdim) # [N, N]
    out = att @ hp + hp                            # [N, D]

Design (transposed-domain, zero on-device transposes, ~54.5 us/exec):
  * Host marshaling only (no input-data compute on host): a is shipped
    TRANSPOSED and cast to bf16 (aT[j,i] = a[i,j]); h is shipped transposed
    in bf16; attention weights are shipped reparameterized/replicated
    (ws2 = W @ w_src tiled to 128 columns, wd2 = W @ w_dst, scalar biases
    b@w_src / b@w_dst + b_att replicated to [128,1]); everything bf16 is
    packed into ONE dram tensor hx = [hT | ws2r | wd2 | W] so one DMA
    covers the whole setup.
  * exp(relu(x)) == max(exp(x), 1) and exp(s_i+t_j) == u_i*v_j with
    u = exp(s) (materialized replicated across partitions, [128, N] bf16,
    via a matmul with the column-replicated ws2) and v = exp(t) landing as
    per-partition scalars ([128, 16]) via 16 one-wide matmuls.
  * Score matrix built directly TRANSPOSED, per 128-row j-chunk:
    PT[j, i] = max(u_i * v_j, 1) * aT[j, i] as two DVE ops (tensor_scalar
    4x-bf16 + tensor_tensor 2x-bf16, ~1.8 us/chunk = the DVE floor); z runs
    3 chunks ahead of the product to fill the pre-loop idle window. aT is
    loaded as 16 per-chunk 0.5 MiB DMAs so chunk j never falsely waits.
  * Matmul: PT chunks are the STATIONARY operand, rhs = [hp0 | 1]
    ([128, 129] bf16, hp0 = h@W bias-free). Output accumulates NATURALLY
    as [i, d] in PSUM and the row-sums fall out as the free 129th column.
    16 accumulators are packed 3-per-bank into 6 PSUM banks. NOTE:
    start=True clears PSUM has_written at BANK granularity on TRN2 - only
    the first accumulator per bank may use it (siblings' first matmul
    relies on the bank-wide clear and overwrites).
  * Finalize: out = psum * (1/rowsum) + hp0 via DVE reciprocal plus, per
    4-chunk group, two fused DVE scalar_tensor_tensors and two
    ACT-scaled-copy + Pool-add pipelines, each group DMA-ing out
    immediately. The missing constant row 2*b_proj (from the bias-free
    rhs/residual) is added back on the host - exact algebra:
    P@hp/rs + hp == P@hp0/rs + hp0 + 2*b_proj.
  * Engines never chain through slow paths: GPSIMD does no bulk
    elementwise (its [128,2048] ops run ~30 us and interlock with DVE perf
    modes); ACT avoids per-chunk work (1x mode + 1.28 us table reload per
    function switch).
"""

import os
import sys

for _p in ("/opt/trn_rl_repo", "/root/.axon_site/_ro/trn_rl_repo"):
    if _p not in sys.path and os.path.isdir(_p):
        sys.path.append(_p)

import numpy as np
from contextlib import ExitStack

import concourse.bass as bass
import concourse.bacc as bacc
import concourse.tile as tile
from concourse import mybir
from concourse.bass_utils import run_bass_kernel_spmd

F32 = mybir.dt.float32
BF16 = mybir.dt.bfloat16
MULT = mybir.AluOpType.mult
MAX = mybir.AluOpType.max
ADD = mybir.AluOpType.add
EXP = mybir.ActivationFunctionType.Exp
LRELU = mybir.ActivationFunctionType.Lrelu
COPY = mybir.ActivationFunctionType.Copy
IDENT = mybir.ActivationFunctionType.Identity

B, N, D = 8, 2048, 128
P = 128           # partitions
NT = N // P       # 16 chunks
N_CORES = 8


def _build_kernel(ctx: ExitStack, tc: tile.TileContext, io: dict):
    nc = tc.nc
    aT = io["aT"]          # [N, N] bf16 dram: aT[j, i] = a[i, j]
    hx_d = io["hx"]        # [D, N+257] bf16: [hT | ws2r | wd2 | W] packed



    cas = io["cas"]        # [128, 1] f32: b_proj @ w_src, replicated
    ba2 = io["ba2"]        # [128, 1] f32: b_proj @ w_dst + b_att, replicated
    out = io["out"]        # [N, D] f32 dram

    cst = ctx.enter_context(tc.tile_pool(name="cst", bufs=1))
    sps = ctx.enter_context(tc.tile_pool(name="sps", bufs=2, space="PSUM"))
    mmp = ctx.enter_context(tc.tile_pool(name="mmp", bufs=1, space="PSUM"))
    zp = ctx.enter_context(tc.tile_pool(name="zp", bufs=1))
    zap = ctx.enter_context(tc.tile_pool(name="zap", bufs=1))
    ppx = ctx.enter_context(tc.tile_pool(name="ppx", bufs=1))
    rp = ctx.enter_context(tc.tile_pool(name="rp", bufs=1))

    # ---- ACT warm-up: trigger the lazy ACT_TABLE_LOAD off the critical path
    warm_in = cst.tile([1, 1], F32)
    nc.vector.memset(warm_in[:], 0.0)
    warm_out = cst.tile([1, 1], F32)
    nc.scalar.activation(warm_out[:], warm_in[:], EXP)

    # ---- critical loads on the Sync HWDGE queue, then the aT flood ----
    hx_sb = cst.tile([P, N + 257], BF16)
    nc.sync.dma_start(hx_sb[:], hx_d[:])
    hTb_sb = hx_sb[:, 0:N]
    ws2rb_sb = hx_sb[:, N:N + 128]
    wd2b_sb = hx_sb[:, N + 128:N + 129]
    Wb_sb = hx_sb[:, N + 129:N + 257]
    cas_sb = cst.tile([P, 1], F32)
    nc.sync.dma_start(cas_sb[:], cas[:])
    aT_tiles = []

    def load_aT(jc):
        a_t = cst.tile([P, N], BF16, tag=f"at{jc}", name=f"aT{jc}")
        nc.sync.dma_start(
            a_t[:],
            aT[jc * P:(jc + 1) * P, :].rearrange("(c p) i -> p c i", p=P))
        aT_tiles.append(a_t)

    for jc in range(NT):
        load_aT(jc)

    # ---- secondary loads on the Scalar HWDGE queue (parallel issue) ----
    ba2_sb = cst.tile([P, 1], F32)
    nc.scalar.dma_start(ba2_sb[:], ba2[:])

    # ---- u_full [p, i] bf16 = exp(s_i) replicated; s = h @ (W w_src) + cas
    u_full = cst.tile([P, N], BF16)
    for s4 in range(4):
        sl = slice(s4 * 512, (s4 + 1) * 512)
        ps = sps.tile([P, 512], F32, tag="sps")
        nc.tensor.matmul(ps[:], ws2rb_sb, hTb_sb[:, sl])
        nc.scalar.activation(u_full[:, sl], ps[:], EXP, bias=cas_sb[:],
                             scale=1.0)

    # ---- t/v: t[jc*128+p] via 1-wide bf16 matmuls on hTb ----
    v_col = cst.tile([P, NT], F32)
    t_ps = sps.tile([P, 512], F32, tag="sps")
    for r in range(NT):
        nc.tensor.matmul(t_ps[:, r:r + 1], hTb_sb[:, r * P:(r + 1) * P],
                         wd2b_sb)
        if r == 7:
            nc.scalar.activation(v_col[:, 0:8], t_ps[:, 0:8], EXP,
                                 bias=ba2_sb[:], scale=1.0)
    nc.scalar.activation(v_col[:, 8:NT], t_ps[:, 8:NT], EXP, bias=ba2_sb[:],
                         scale=1.0)

    # ---- hp chunks: hp0 = h @ W (f32), hp_aug = [hp0 | 1] bf16.
    # Algebra: P@hp0/rs + hp0 == P@hp/rs + hp - 2*b_proj; the constant
    # 2*b_proj row is added back on the HOST after the kernel returns. ----
    hp_aug = cst.tile([P, NT, 132], BF16)
    nc.vector.memset(hp_aug[:, :, 128:129], 1.0)

    def emit_hp(r):
        if r >= NT:
            return
        ps = sps.tile([P, 512], F32, tag="sps", name="hp_ps")
        nc.tensor.matmul(ps[:, :P], hTb_sb[:, r * P:(r + 1) * P], Wb_sb)
        nc.scalar.activation(hp_aug[:, r, 0:P], ps[:, :P], COPY)

    for r in range(NT):
        emit_hp(r)

    # ---- main psum: 16 accumulators [128, 129], packed 3 per bank ----
    mm_tiles = [mmp.tile([P, 512], F32, tag=f"mm{b}", name=f"mm{b}")
                for b in range(6)]

    def acc(ic):
        bank, slot = divmod(ic, 3)
        off = 130 * slot
        return mm_tiles[bank], off

    out_stage = cst.tile([P, NT, D], F32)

    # ---- main loop over j-chunks; z runs 3 chunks ahead of the product
    # so the pre-p0 DVE idle window does useful z work ----
    z_tiles = {}

    def emit_z(jc):
        if jc >= NT:
            return
        z_t = zp.tile([P, N], BF16, tag=f"z{jc % 3}", name="z_t")
        nc.vector.tensor_scalar(z_t[:], u_full[:], v_col[:, jc:jc + 1],
                                1.0, MULT, MAX)
        z_tiles[jc] = z_t

    for jc in range(3):
        emit_z(jc)
    for jc in range(NT):
        p_t = ppx.tile([P, N], BF16, tag=f"p{jc % 3}", name="p_t")
        nc.vector.tensor_tensor(p_t[:], z_tiles.pop(jc)[:], aT_tiles[jc][:],
                                MULT)
        emit_z(jc + 3)
        for ic in range(NT):
            mt, off = acc(ic)
            # start=True clears PSUM has_written at BANK granularity on
            # TRN2: only the first accumulator of each bank may use it, or
            # it wipes its siblings' jc=0 contribution. The bank-wide clear
            # leaves the sibling slots "unwritten", so their jc=0 matmul
            # (start=False) overwrites rather than accumulates - correct on
            # every execution.
            nc.tensor.matmul(mt[:, off:off + 129],
                             p_t[:, ic * P:(ic + 1) * P],
                             hp_aug[:, jc, 0:129],
                             start=(jc == 0 and ic % 3 == 0),
                             stop=(jc == NT - 1),
                             skip_group_check=True)

    # ---- finalize in 4 groups of 4: out = psum/rowsum + hp0.
    # Two parallel pipelines per group: 2 chunks fused on DVE (stt), 2 on
    # ACT (scaled copy) + Pool (residual add), so no engine chain
    # serializes. ----
    out_r = out.rearrange("(r p) d -> p r d", p=P)
    rinv_all = cst.tile([P, NT], F32)
    for g in range(4):
        ics = list(range(4 * g, 4 * g + 4))
        for ic in ics:
            mt, off = acc(ic)
            nc.vector.reciprocal(rinv_all[:, ic:ic + 1],
                                 mt[:, off + 128:off + 129])
        for ic in ics[2:]:
            mt, off = acc(ic)
            fin = rp.tile([P, D], F32, tag=f"fin{ic % 4}", name="fin")
            nc.scalar.activation(fin[:], mt[:, off:off + 128], COPY,
                                 scale=rinv_all[:, ic:ic + 1])
            nc.gpsimd.tensor_tensor(out_stage[:, ic, :], fin[:],
                                    hp_aug[:, ic, 0:P], ADD)
        for ic in ics[:2]:
            mt, off = acc(ic)
            nc.vector.scalar_tensor_tensor(out_stage[:, ic, :],
                                           mt[:, off:off + 128],
                                           rinv_all[:, ic:ic + 1],
                                           hp_aug[:, ic, 0:P], MULT, ADD)
        nc.sync.dma_start(out_r[:, 4 * g:4 * g + 4, :],
                          out_stage[:, 4 * g:4 * g + 4, :])


_CACHE = {}


def _get_compiled():
    if "nc" in _CACHE:
        return _CACHE["nc"], _CACHE["names"]

    nc = bacc.Bacc("TRN2", target_bir_lowering=False, debug=False)
    io = {}
    io["aT"] = nc.dram_tensor("aT", [N, N], BF16, kind="ExternalInput").ap()
    io["hx"] = nc.dram_tensor("hx", [D, N + 257], BF16, kind="ExternalInput").ap()

    io["cas"] = nc.dram_tensor("cas", [P, 1], F32, kind="ExternalInput").ap()
    io["ba2"] = nc.dram_tensor("ba2", [P, 1], F32, kind="ExternalInput").ap()
    io["out"] = nc.dram_tensor("out", [N, D], F32, kind="ExternalOutput").ap()

    with tile.TileContext(nc) as tc:
        with ExitStack() as ctx:
            _build_kernel(ctx, tc, io)
    nc.compile()

    _CACHE["nc"] = nc
    _CACHE["names"] = list(io.keys())
    return nc, _CACHE["names"]


def _make_in_maps(a, h, W_proj, b_proj, w_att, b_att):
    import ml_dtypes
    bf16 = ml_dtypes.bfloat16

    a = np.asarray(a, dtype=np.float32)
    h = np.asarray(h, dtype=np.float32)
    W_proj = np.ascontiguousarray(W_proj, dtype=np.float32)
    b_proj = np.asarray(b_proj, dtype=np.float32).reshape(D)
    w_att = np.ascontiguousarray(w_att, dtype=np.float32)
    w_src, w_dst = w_att[:D], w_att[D:]

    ws2 = (W_proj @ w_src).astype(np.float32)
    ws2rb = np.tile(ws2[:, None], (1, P)).astype(bf16)
    wd2b = (W_proj @ w_dst).astype(np.float32).reshape(D, 1).astype(bf16)
    cas = np.full((P, 1), float(b_proj @ w_src), dtype=np.float32)
    ba2 = np.full((P, 1), float(b_proj @ w_dst) + float(b_att),
                  dtype=np.float32)

    in_maps = []
    for c in range(N_CORES):
        hT_c = np.ascontiguousarray(h[c].T)
        in_maps.append({
            "aT": np.ascontiguousarray(a[c].T).astype(bf16),
            "hx": np.ascontiguousarray(np.concatenate(
                [hT_c.astype(bf16), ws2rb, wd2b, W_proj.astype(bf16)],
                axis=1)),
            "cas": cas, "ba2": ba2,
        })
    return in_maps


def _get_executable():
    """Build (once) a sharded PJRT callable for the compiled Bass module.

    Mirrors concourse.bass2jax.run_bass_via_pjrt but keeps the jitted
    function so repeated calls don't retrace/recompile.
    """
    if "exe" in _CACHE:
        return _CACHE["exe"]

    import jax
    from jax.sharding import Mesh, PartitionSpec
    from jax.experimental.shard_map import shard_map
    from concourse import bass2jax, mybir as _mybir

    nc, _ = _get_compiled()
    bass2jax.install_neuronx_cc_hook()

    partition_name = (nc.partition_id_tensor.name
                      if nc.partition_id_tensor else None)
    in_names, out_names, out_avals, zero_outs = [], [], [], []
    for alloc in nc.m.functions[0].allocations:
        if not isinstance(alloc, _mybir.MemoryLocationSet):
            continue
        name = alloc.memorylocations[0].name
        if alloc.kind == "ExternalInput":
            if name != partition_name:
                in_names.append(name)
        elif alloc.kind == "ExternalOutput":
            shape = tuple(alloc.tensor_shape)
            dtype = _mybir.dt.np(alloc.dtype)
            out_names.append(name)
            out_avals.append(jax.core.ShapedArray(shape, dtype))
            zero_outs.append(np.zeros(shape, dtype))
    n_params = len(in_names)
    n_outs = len(out_avals)
    all_in_names = in_names + out_names + (
        [partition_name] if partition_name else [])
    donate = tuple(range(n_params, n_params + n_outs))

    def _body(*args):
        operands = list(args)
        if partition_name is not None:
            operands.append(bass2jax.partition_id_tensor())
        outs = bass2jax._bass_exec_p.bind(
            *operands,
            out_avals=tuple(out_avals),
            in_names=tuple(all_in_names),
            out_names=tuple(out_names),
            lowering_input_output_aliases=(),
            sim_require_finite=True,
            sim_require_nnan=True,
            nc=nc,
        )
        return tuple(outs)

    devices = jax.devices()[:N_CORES]
    mesh = Mesh(np.asarray(devices), ("core",))
    in_specs = (PartitionSpec("core"),) * (n_params + n_outs)
    out_specs = (PartitionSpec("core"),) * n_outs
    fn = jax.jit(
        shard_map(_body, mesh=mesh, in_specs=in_specs, out_specs=out_specs,
                  check_rep=False),
        donate_argnums=donate, keep_unused=True,
    )
    exe = {
        "fn": fn, "mesh": mesh, "in_names": in_names,
        "out_names": out_names, "out_avals": out_avals,
        "zero_outs": zero_outs, "n_params": n_params,
    }
    _CACHE["exe"] = exe
    return exe


def _concat_inputs(exe, in_maps):
    return [
        np.concatenate([np.asarray(in_maps[c][name])
                        for c in range(N_CORES)], axis=0)
        for name in exe["in_names"]
    ]


def _concat_zeros(exe):
    return [np.zeros((N_CORES * z.shape[0], *z.shape[1:]), z.dtype)
            for z in exe["zero_outs"]]


def kernel(a, h, W_proj, b_proj, w_att, b_att):
    exe = _get_executable()
    in_maps = _make_in_maps(a, h, W_proj, b_proj, w_att, b_att)
    out_arrs = exe["fn"](*_concat_inputs(exe, in_maps), *_concat_zeros(exe))
    i = exe["out_names"].index("out")
    out = np.asarray(out_arrs[i]).reshape(N_CORES, N, D).copy()
    # the kernel computes P@hp0/rs + hp0 with hp0 = h@W (bias-free); the
    # missing constant row 2*b_proj is added here (exact algebra).
    out += 2.0 * np.asarray(b_proj, dtype=np.float32).reshape(1, 1, D)
    return out


if __name__ == "__main__":
    rng = np.random.default_rng(0)
    a = rng.random((B, N, N), dtype=np.float32)
    h = rng.standard_normal((B, N, D), dtype=np.float32)
    W_proj = (rng.standard_normal((D, D)) / np.sqrt(D)).astype(np.float32)
    b_proj = (rng.standard_normal(D) * 0.01).astype(np.float32)
    w_att = (rng.standard_normal(2 * D) / np.sqrt(2 * D)).astype(np.float32)
    b_att = np.float32(rng.standard_normal() * 0.01)

    got = kernel(a=a, h=h, W_proj=W_proj, b_proj=b_proj, w_att=w_att,
                 b_att=b_att)

    hp = h @ W_proj + b_proj
    s = hp @ w_att[:D]
    t = hp @ w_att[D:]
    e = np.maximum(s[:, :, None] + t[:, None, :] + b_att, 0.0)
    att = np.exp(e) * a
    att = att / att.sum(-1, keepdims=True)
    ref = att @ hp + hp

    err = np.abs(got - ref).max() / np.abs(ref).max()
    print("rel err:", err)


# revision 27
# speedup vs baseline: 1.0423x; 1.0423x over previous
"""GAT layer kernel for Trainium2, data-parallel over batch across 8 NeuronCores.

Per batch element b (one core each):
    hp  = h @ W_proj + b_proj                      # [N, D]
    s   = hp @ w_src ; t = hp @ w_dst              # [N]
    e   = relu(s[:,None] + t[None,:] + b_att)      # [N, N]
    att = exp(e) * a ; att /= att.sum(-1, keepdim) # [N, N]
    out = att @ hp + hp                            # [N, D]

Design (transposed-domain, zero on-device transposes, ~54.5 us/exec):
  * Host marshaling only (no input-data compute on host): a is shipped
    TRANSPOSED and cast to bf16 (aT[j,i] = a[i,j]); h is shipped transposed
    in bf16; attention weights are shipped reparameterized/replicated
    (ws2 = W @ w_src tiled to 128 columns, wd2 = W @ w_dst, scalar biases
    b@w_src / b@w_dst + b_att replicated to [128,1]); everything bf16 is
    packed into ONE dram tensor hx = [hT | ws2r | wd2 | W] so one DMA
    covers the whole setup.
  * exp(relu(x)) == max(exp(x), 1) and exp(s_i+t_j) == u_i*v_j with
    u = exp(s) (materialized replicated across partitions, [128, N] bf16,
    via a matmul with the column-replicated ws2) and v = exp(t) landing as
    per-partition scalars ([128, 16]) via 16 one-wide matmuls.
  * Score matrix built directly TRANSPOSED, per 128-row j-chunk:
    PT[j, i] = max(u_i * v_j, 1) * aT[j, i] as two DVE ops (tensor_scalar
    4x-bf16 + tensor_tensor 2x-bf16, ~1.8 us/chunk = the DVE floor); z runs
    3 chunks ahead of the product to fill the pre-loop idle window. aT is
    loaded as 16 per-chunk 0.5 MiB DMAs so chunk j never falsely waits.
  * Matmul: PT chunks are the STATIONARY operand, rhs = [hp0 | 1]
    ([128, 129] bf16, hp0 = h@W bias-free). Output accumulates NATURALLY
    as [i, d] in PSUM and the row-sums fall out as the free 129th column.
    16 accumulators are packed 3-per-bank into 6 PSUM banks. NOTE:
    start=True clears PSUM has_written at BANK granularity on TRN2 - only
    the first accumulator per bank may use it (siblings' first matmul
    relies on the bank-wide clear and overwrites).
  * Finalize: out = psum * (1/rowsum) + hp0 via DVE reciprocal plus, per
    4-chunk group, two fused DVE scalar_tensor_tensors and two
    ACT-scaled-copy + Pool-add pipelines, each group DMA-ing out
    immediately. The missing constant row 2*b_proj (from the bias-free
    rhs/residual) is added back on the host - exact algebra:
    P@hp/rs + hp == P@hp0/rs + hp0 + 2*b_proj.
  * Engines never chain through slow paths: GPSIMD does no bulk
    elementwise (its [128,2048] ops run ~30 us and interlock with DVE perf
    modes); ACT avoids per-chunk work (1x mode + 1.28 us table reload per
    function switch).
"""

import os
import sys

for _p in ("/opt/trn_rl_repo", "/root/.axon_site/_ro/trn_rl_repo"):
    if _p not in sys.path and os.path.isdir(_p):
        sys.path.append(_p)

import numpy as np
from contextlib import ExitStack

import concourse.bass as bass
import concourse.bacc as bacc
import concourse.tile as tile
from concourse import mybir
from concourse.bass_utils import run_bass_kernel_spmd

F32 = mybir.dt.float32
BF16 = mybir.dt.bfloat16
MULT = mybir.AluOpType.mult
MAX = mybir.AluOpType.max
ADD = mybir.AluOpType.add
EXP = mybir.ActivationFunctionType.Exp
LRELU = mybir.ActivationFunctionType.Lrelu
COPY = mybir.ActivationFunctionType.Copy
IDENT = mybir.ActivationFunctionType.Identity

B, N, D = 8, 2048, 128
P = 128           # partitions
NT = N // P       # 16 chunks
N_CORES = 8


def _build_kernel(ctx: ExitStack, tc: tile.TileContext, io: dict):
    nc = tc.nc
    aT = io["aT"]          # [N, N] bf16 dram: aT[j, i] = a[i, j]
    hx_d = io["hx"]        # [D, N+257] bf16: [hT | ws2r | wd2 | W] packed



    cas = io["cas"]        # [128, 1] f32: b_proj @ w_src, replicated
    ba2 = io["ba2"]        # [128, 1] f32: b_proj @ w_dst + b_att, replicated
    out = io["out"]        # [N, D] f32 dram

    cst = ctx.enter_context(tc.tile_pool(name="cst", bufs=1))
    sps = ctx.enter_context(tc.tile_pool(name="sps", bufs=2, space="PSUM"))
    mmp = ctx.enter_context(tc.tile_pool(name="mmp", bufs=1, space="PSUM"))
    zp = ctx.enter_context(tc.tile_pool(name="zp", bufs=1))
    zap = ctx.enter_context(tc.tile_pool(name="zap", bufs=1))
    ppx = ctx.enter_context(tc.tile_pool(name="ppx", bufs=1))
    rp = ctx.enter_context(tc.tile_pool(name="rp", bufs=1))

    # ---- ACT warm-up: trigger the lazy ACT_TABLE_LOAD off the critical path
    warm_in = cst.tile([1, 1], F32)
    nc.vector.memset(warm_in[:], 0.0)
    warm_out = cst.tile([1, 1], F32)
    nc.scalar.activation(warm_out[:], warm_in[:], EXP)

    # ---- critical loads on the Sync HWDGE queue, then the aT flood ----
    hx_sb = cst.tile([P, N + 257], BF16)
    nc.sync.dma_start(hx_sb[:, N:N + 257], hx_d[:, N:N + 257])
    nc.sync.dma_start(hx_sb[:, 0:512], hx_d[:, 0:512])
    nc.sync.dma_start(hx_sb[:, 512:N], hx_d[:, 512:N])
    hTb_sb = hx_sb[:, 0:N]
    ws2rb_sb = hx_sb[:, N:N + 128]
    wd2b_sb = hx_sb[:, N + 128:N + 129]
    Wb_sb = hx_sb[:, N + 129:N + 257]
    cas_sb = cst.tile([P, 1], F32)
    nc.sync.dma_start(cas_sb[:], cas[:])
    aT_tiles = []

    def load_aT(jc):
        a_t = cst.tile([P, N], BF16, tag=f"at{jc}", name=f"aT{jc}")
        nc.sync.dma_start(
            a_t[:],
            aT[jc * P:(jc + 1) * P, :].rearrange("(c p) i -> p c i", p=P))
        aT_tiles.append(a_t)

    for jc in range(NT):
        load_aT(jc)

    # ---- secondary loads on the Scalar HWDGE queue (parallel issue) ----
    ba2_sb = cst.tile([P, 1], F32)
    nc.scalar.dma_start(ba2_sb[:], ba2[:])

    # ---- u_full [p, i] bf16 = exp(s_i) replicated; s = h @ (W w_src) + cas
    u_full = cst.tile([P, N], BF16)
    for s4 in range(4):
        sl = slice(s4 * 512, (s4 + 1) * 512)
        ps = sps.tile([P, 512], F32, tag="sps")
        nc.tensor.matmul(ps[:], ws2rb_sb, hTb_sb[:, sl])
        nc.scalar.activation(u_full[:, sl], ps[:], EXP, bias=cas_sb[:],
                             scale=1.0)

    # ---- t/v: t[jc*128+p] via 1-wide bf16 matmuls on hTb ----
    v_col = cst.tile([P, NT], F32)
    t_ps = sps.tile([P, 512], F32, tag="sps")
    for r in range(NT):
        nc.tensor.matmul(t_ps[:, r:r + 1], hTb_sb[:, r * P:(r + 1) * P],
                         wd2b_sb)
        if r == 7:
            nc.scalar.activation(v_col[:, 0:8], t_ps[:, 0:8], EXP,
                                 bias=ba2_sb[:], scale=1.0)
    nc.scalar.activation(v_col[:, 8:NT], t_ps[:, 8:NT], EXP, bias=ba2_sb[:],
                         scale=1.0)

    # ---- hp chunks: hp0 = h @ W (f32), hp_aug = [hp0 | 1] bf16.
    # Algebra: P@hp0/rs + hp0 == P@hp/rs + hp - 2*b_proj; the constant
    # 2*b_proj row is added back on the HOST after the kernel returns. ----
    hp_aug = cst.tile([P, NT, 132], BF16)
    nc.vector.memset(hp_aug[:, :, 128:129], 1.0)

    def emit_hp(r):
        if r >= NT:
            return
        ps = sps.tile([P, 512], F32, tag="sps", name="hp_ps")
        nc.tensor.matmul(ps[:, :P], hTb_sb[:, r * P:(r + 1) * P], Wb_sb)
        nc.scalar.activation(hp_aug[:, r, 0:P], ps[:, :P], COPY)

    for r in range(NT):
        emit_hp(r)

    # ---- main psum: 16 accumulators [128, 129], packed 3 per bank ----
    mm_tiles = [mmp.tile([P, 512], F32, tag=f"mm{b}", name=f"mm{b}")
                for b in range(6)]

    def acc(ic):
        bank, slot = divmod(ic, 3)
        off = 130 * slot
        return mm_tiles[bank], off

    out_stage = cst.tile([P, NT, D], F32)

    # ---- main loop over j-chunks; z runs 3 chunks ahead of the product
    # so the pre-p0 DVE idle window does useful z work ----
    z_tiles = {}

    def emit_z(jc):
        if jc >= NT:
            return
        z_t = zp.tile([P, N], BF16, tag=f"z{jc % 3}", name="z_t")
        nc.vector.tensor_scalar(z_t[:], u_full[:], v_col[:, jc:jc + 1],
                                1.0, MULT, MAX)
        z_tiles[jc] = z_t

    for jc in range(3):
        emit_z(jc)
    for jc in range(NT):
        p_t = ppx.tile([P, N], BF16, tag=f"p{jc % 3}", name="p_t")
        nc.vector.tensor_tensor(p_t[:], z_tiles.pop(jc)[:], aT_tiles[jc][:],
                                MULT)
        emit_z(jc + 3)
        for ic in range(NT):
            mt, off = acc(ic)
            # start=True clears PSUM has_written at BANK granularity on
            # TRN2: only the first accumulator of each bank may use it, or
            # it wipes its siblings' jc=0 contribution. The bank-wide clear
            # leaves the sibling slots "unwritten", so their jc=0 matmul
            # (start=False) overwrites rather than accumulates - correct on
            # every execution.
            nc.tensor.matmul(mt[:, off:off + 129],
                             p_t[:, ic * P:(ic + 1) * P],
                             hp_aug[:, jc, 0:129],
                             start=(jc == 0 and ic % 3 == 0),
                             stop=(jc == NT - 1),
                             skip_group_check=True)

    # ---- finalize in 4 groups of 4: out = psum/rowsum + hp0.
    # Two parallel pipelines per group: 2 chunks fused on DVE (stt), 2 on
    # ACT (scaled copy) + Pool (residual add), so no engine chain
    # serializes. ----
    out_r = out.rearrange("(r p) d -> p r d", p=P)
    rinv_all = cst.tile([P, NT], F32)
    for g in range(4):
        ics = list(range(4 * g, 4 * g + 4))
        for ic in ics:
            mt, off = acc(ic)
            nc.vector.reciprocal(rinv_all[:, ic:ic + 1],
                                 mt[:, off + 128:off + 129])
        for ic in ics[2:]:
            mt, off = acc(ic)
            fin = rp.tile([P, D], F32, tag=f"fin{ic % 4}", name="fin")
            nc.scalar.activation(fin[:], mt[:, off:off + 128], COPY,
                                 scale=rinv_all[:, ic:ic + 1])
            nc.gpsimd.tensor_tensor(out_stage[:, ic, :], fin[:],
                                    hp_aug[:, ic, 0:P], ADD)
        for ic in ics[:2]:
            mt, off = acc(ic)
            nc.vector.scalar_tensor_tensor(out_stage[:, ic, :],
                                           mt[:, off:off + 128],
                                           rinv_all[:, ic:ic + 1],
                                           hp_aug[:, ic, 0:P], MULT, ADD)
        nc.sync.dma_start(out_r[:, 4 * g:4 * g + 4, :],
                          out_stage[:, 4 * g:4 * g + 4, :])


_CACHE = {}


def _get_compiled():
    if "nc" in _CACHE:
        return _CACHE["nc"], _CACHE["names"]

    nc = bacc.Bacc("TRN2", target_bir_lowering=False, debug=False)
    io = {}
    io["aT"] = nc.dram_tensor("aT", [N, N], BF16, kind="ExternalInput").ap()
    io["hx"] = nc.dram_tensor("hx", [D, N + 257], BF16, kind="ExternalInput").ap()

    io["cas"] = nc.dram_tensor("cas", [P, 1], F32, kind="ExternalInput").ap()
    io["ba2"] = nc.dram_tensor("ba2", [P, 1], F32, kind="ExternalInput").ap()
    io["out"] = nc.dram_tensor("out", [N, D], F32, kind="ExternalOutput").ap()

    with tile.TileContext(nc) as tc:
        with ExitStack() as ctx:
            _build_kernel(ctx, tc, io)
    nc.compile()

    _CACHE["nc"] = nc
    _CACHE["names"] = list(io.keys())
    return nc, _CACHE["names"]


def _make_in_maps(a, h, W_proj, b_proj, w_att, b_att):
    import ml_dtypes
    bf16 = ml_dtypes.bfloat16

    a = np.asarray(a, dtype=np.float32)
    h = np.asarray(h, dtype=np.float32)
    W_proj = np.ascontiguousarray(W_proj, dtype=np.float32)
    b_proj = np.asarray(b_proj, dtype=np.float32).reshape(D)
    w_att = np.ascontiguousarray(w_att, dtype=np.float32)
    w_src, w_dst = w_att[:D], w_att[D:]

    ws2 = (W_proj @ w_src).astype(np.float32)
    ws2rb = np.tile(ws2[:, None], (1, P)).astype(bf16)
    wd2b = (W_proj @ w_dst).astype(np.float32).reshape(D, 1).astype(bf16)
    cas = np.full((P, 1), float(b_proj @ w_src), dtype=np.float32)
    ba2 = np.full((P, 1), float(b_proj @ w_dst) + float(b_att),
                  dtype=np.float32)

    in_maps = []
    for c in range(N_CORES):
        hT_c = np.ascontiguousarray(h[c].T)
        in_maps.append({
            "aT": np.ascontiguousarray(a[c].T).astype(bf16),
            "hx": np.ascontiguousarray(np.concatenate(
                [hT_c.astype(bf16), ws2rb, wd2b, W_proj.astype(bf16)],
                axis=1)),
            "cas": cas, "ba2": ba2,
        })
    return in_maps


def _get_executable():
    """Build (once) a sharded PJRT callable for the compiled Bass module.

    Mirrors concourse.bass2jax.run_bass_via_pjrt but keeps the jitted
    function so repeated calls don't retrace/recompile.
    """
    if "exe" in _CACHE:
        return _CACHE["exe"]

    import jax
    from jax.sharding import Mesh, PartitionSpec
    from jax.experimental.shard_map import shard_map
    from concourse import bass2jax, mybir as _mybir

    nc, _ = _get_compiled()
    bass2jax.install_neuronx_cc_hook()

    partition_name = (nc.partition_id_tensor.name
                      if nc.partition_id_tensor else None)
    in_names, out_names, out_avals, zero_outs = [], [], [], []
    for alloc in nc.m.functions[0].allocations:
        if not isinstance(alloc, _mybir.MemoryLocationSet):
            continue
        name = alloc.memorylocations[0].name
        if alloc.kind == "ExternalInput":
            if name != partition_name:
                in_names.append(name)
        elif alloc.kind == "ExternalOutput":
            shape = tuple(alloc.tensor_shape)
            dtype = _mybir.dt.np(alloc.dtype)
            out_names.append(name)
            out_avals.append(jax.core.ShapedArray(shape, dtype))
            zero_outs.append(np.zeros(shape, dtype))
    n_params = len(in_names)
    n_outs = len(out_avals)
    all_in_names = in_names + out_names + (
        [partition_name] if partition_name else [])
    donate = tuple(range(n_params, n_params + n_outs))

    def _body(*args):
        operands = list(args)
        if partition_name is not None:
            operands.append(bass2jax.partition_id_tensor())
        outs = bass2jax._bass_exec_p.bind(
            *operands,
            out_avals=tuple(out_avals),
            in_names=tuple(all_in_names),
            out_names=tuple(out_names),
            lowering_input_output_aliases=(),
            sim_require_finite=True,
            sim_require_nnan=True,
            nc=nc,
        )
        return tuple(outs)

    devices = jax.devices()[:N_CORES]
    mesh = Mesh(np.asarray(devices), ("core",))
    in_specs = (PartitionSpec("core"),) * (n_params + n_outs)
    out_specs = (PartitionSpec("core"),) * n_outs
    fn = jax.jit(
        shard_map(_body, mesh=mesh, in_specs=in_specs, out_specs=out_specs,
                  check_rep=False),
        donate_argnums=donate, keep_unused=True,
    )
    exe = {
        "fn": fn, "mesh": mesh, "in_names": in_names,
        "out_names": out_names, "out_avals": out_avals,
        "zero_outs": zero_outs, "n_params": n_params,
    }
    _CACHE["exe"] = exe
    return exe


def _concat_inputs(exe, in_maps):
    return [
        np.concatenate([np.asarray(in_maps[c][name])
                        for c in range(N_CORES)], axis=0)
        for name in exe["in_names"]
    ]


def _concat_zeros(exe):
    return [np.zeros((N_CORES * z.shape[0], *z.shape[1:]), z.dtype)
            for z in exe["zero_outs"]]


def kernel(a, h, W_proj, b_proj, w_att, b_att):
    exe = _get_executable()
    in_maps = _make_in_maps(a, h, W_proj, b_proj, w_att, b_att)
    out_arrs = exe["fn"](*_concat_inputs(exe, in_maps), *_concat_zeros(exe))
    i = exe["out_names"].index("out")
    out = np.asarray(out_arrs[i]).reshape(N_CORES, N, D).copy()
    # the kernel computes P@hp0/rs + hp0 with hp0 = h@W (bias-free); the
    # missing constant row 2*b_proj is added here (exact algebra).
    out += 2.0 * np.asarray(b_proj, dtype=np.float32).reshape(1, 1, D)
    return out


if __name__ == "__main__":
    rng = np.random.default_rng(0)
    a = rng.random((B, N, N), dtype=np.float32)
    h = rng.standard_normal((B, N, D), dtype=np.float32)
    W_proj = (rng.standard_normal((D, D)) / np.sqrt(D)).astype(np.float32)
    b_proj = (rng.standard_normal(D) * 0.01).astype(np.float32)
    w_att = (rng.standard_normal(2 * D) / np.sqrt(2 * D)).astype(np.float32)
    b_att = np.float32(rng.standard_normal() * 0.01)

    got = kernel(a=a, h=h, W_proj=W_proj, b_proj=b_proj, w_att=w_att,
                 b_att=b_att)

    hp = h @ W_proj + b_proj
    s = hp @ w_att[:D]
    t = hp @ w_att[D:]
    e = np.maximum(s[:, :, None] + t[:, None, :] + b_att, 0.0)
    att = np.exp(e) * a
    att = att / att.sum(-1, keepdims=True)
    ref = att @ hp + hp

    err = np.abs(got - ref).max() / np.abs(ref).max()
    print("rel err:", err)


# revision 28
# speedup vs baseline: 1.0512x; 1.0085x over previous
"""GAT layer kernel for Trainium2, data-parallel over batch across 8 NeuronCores.

Per batch element b (one core each):
    hp  = h @ W_proj + b_proj                      # [N, D]
    s   = hp @ w_src ; t = hp @ w_dst              # [N]
    e   = relu(s[:,None] + t[None,:] + b_att)      # [N, N]
    att = exp(e) * a ; att /= att.sum(-1, keepdim) # [N, N]
    out = att @ hp + hp                            # [N, D]

Design (transposed-domain, zero on-device transposes, ~54.5 us/exec):
  * Host marshaling only (no input-data compute on host): a is shipped
    TRANSPOSED and cast to bf16 (aT[j,i] = a[i,j]); h is shipped transposed
    in bf16; attention weights are shipped reparameterized/replicated
    (ws2 = W @ w_src tiled to 128 columns, wd2 = W @ w_dst, scalar biases
    b@w_src / b@w_dst + b_att replicated to [128,1]); everything bf16 is
    packed into ONE dram tensor hx = [hT | ws2r | wd2 | W] so one DMA
    covers the whole setup.
  * exp(relu(x)) == max(exp(x), 1) and exp(s_i+t_j) == u_i*v_j with
    u = exp(s) (materialized replicated across partitions, [128, N] bf16,
    via a matmul with the column-replicated ws2) and v = exp(t) landing as
    per-partition scalars ([128, 16]) via 16 one-wide matmuls.
  * Score matrix built directly TRANSPOSED, per 128-row j-chunk:
    PT[j, i] = max(u_i * v_j, 1) * aT[j, i] as two DVE ops (tensor_scalar
    4x-bf16 + tensor_tensor 2x-bf16, ~1.8 us/chunk = the DVE floor); z runs
    3 chunks ahead of the product to fill the pre-loop idle window. aT is
    loaded as 16 per-chunk 0.5 MiB DMAs so chunk j never falsely waits.
  * Matmul: PT chunks are the STATIONARY operand, rhs = [hp0 | 1]
    ([128, 129] bf16, hp0 = h@W bias-free). Output accumulates NATURALLY
    as [i, d] in PSUM and the row-sums fall out as the free 129th column.
    16 accumulators are packed 3-per-bank into 6 PSUM banks. NOTE:
    start=True clears PSUM has_written at BANK granularity on TRN2 - only
    the first accumulator per bank may use it (siblings' first matmul
    relies on the bank-wide clear and overwrites).
  * Finalize: out = psum * (1/rowsum) + hp0 via DVE reciprocal plus, per
    4-chunk group, two fused DVE scalar_tensor_tensors and two
    ACT-scaled-copy + Pool-add pipelines, each group DMA-ing out
    immediately. The missing constant row 2*b_proj (from the bias-free
    rhs/residual) is added back on the host - exact algebra:
    P@hp/rs + hp == P@hp0/rs + hp0 + 2*b_proj.
  * Engines never chain through slow paths: GPSIMD does no bulk
    elementwise (its [128,2048] ops run ~30 us and interlock with DVE perf
    modes); ACT avoids per-chunk work (1x mode + 1.28 us table reload per
    function switch).
"""

import os
import sys

for _p in ("/opt/trn_rl_repo", "/root/.axon_site/_ro/trn_rl_repo"):
    if _p not in sys.path and os.path.isdir(_p):
        sys.path.append(_p)

import numpy as np
from contextlib import ExitStack

import concourse.bass as bass
import concourse.bacc as bacc
import concourse.tile as tile
from concourse import mybir
from concourse.bass_utils import run_bass_kernel_spmd

F32 = mybir.dt.float32
BF16 = mybir.dt.bfloat16
MULT = mybir.AluOpType.mult
MAX = mybir.AluOpType.max
ADD = mybir.AluOpType.add
EXP = mybir.ActivationFunctionType.Exp
LRELU = mybir.ActivationFunctionType.Lrelu
COPY = mybir.ActivationFunctionType.Copy
IDENT = mybir.ActivationFunctionType.Identity

B, N, D = 8, 2048, 128
P = 128           # partitions
NT = N // P       # 16 chunks
N_CORES = 8


def _build_kernel(ctx: ExitStack, tc: tile.TileContext, io: dict):
    nc = tc.nc
    aT = io["aT"]          # [N, N] bf16 dram: aT[j, i] = a[i, j]
    hx_d = io["hx"]        # [D, N+257] bf16: [hT | ws2r | wd2 | W] packed



    cas = io["cas"]        # [128, 1] f32: b_proj @ w_src, replicated
    ba2 = io["ba2"]        # [128, 1] f32: b_proj @ w_dst + b_att, replicated
    out = io["out"]        # [P, NT*D] f32 dram, host-rearranged to [N, D]

    cst = ctx.enter_context(tc.tile_pool(name="cst", bufs=1))
    sps = ctx.enter_context(tc.tile_pool(name="sps", bufs=2, space="PSUM"))
    mmp = ctx.enter_context(tc.tile_pool(name="mmp", bufs=1, space="PSUM"))
    zp = ctx.enter_context(tc.tile_pool(name="zp", bufs=1))
    zap = ctx.enter_context(tc.tile_pool(name="zap", bufs=1))
    ppx = ctx.enter_context(tc.tile_pool(name="ppx", bufs=1))
    rp = ctx.enter_context(tc.tile_pool(name="rp", bufs=1))

    # ---- ACT warm-up: trigger the lazy ACT_TABLE_LOAD off the critical path
    warm_in = cst.tile([1, 1], F32)
    nc.vector.memset(warm_in[:], 0.0)
    warm_out = cst.tile([1, 1], F32)
    nc.scalar.activation(warm_out[:], warm_in[:], EXP)

    # ---- critical loads on the Sync HWDGE queue, then the aT flood ----
    hx_sb = cst.tile([P, N + 257], BF16)
    nc.sync.dma_start(hx_sb[:, N:N + 257], hx_d[:, N:N + 257])
    nc.sync.dma_start(hx_sb[:, 0:512], hx_d[:, 0:512])
    nc.sync.dma_start(hx_sb[:, 512:N], hx_d[:, 512:N])
    hTb_sb = hx_sb[:, 0:N]
    ws2rb_sb = hx_sb[:, N:N + 128]
    wd2b_sb = hx_sb[:, N + 128:N + 129]
    Wb_sb = hx_sb[:, N + 129:N + 257]
    cas_sb = cst.tile([P, 1], F32)
    nc.sync.dma_start(cas_sb[:], cas[:])
    aT_tiles = []

    def load_aT(jc):
        a_t = cst.tile([P, N], BF16, tag=f"at{jc}", name=f"aT{jc}")
        nc.sync.dma_start(
            a_t[:],
            aT[jc * P:(jc + 1) * P, :].rearrange("(c p) i -> p c i", p=P))
        aT_tiles.append(a_t)

    for jc in range(NT):
        load_aT(jc)

    # ---- secondary loads on the Scalar HWDGE queue (parallel issue) ----
    ba2_sb = cst.tile([P, 1], F32)
    nc.scalar.dma_start(ba2_sb[:], ba2[:])

    # ---- u_full [p, i] bf16 = exp(s_i) replicated; s = h @ (W w_src) + cas
    u_full = cst.tile([P, N], BF16)
    for s4 in range(4):
        sl = slice(s4 * 512, (s4 + 1) * 512)
        ps = sps.tile([P, 512], F32, tag="sps")
        nc.tensor.matmul(ps[:], ws2rb_sb, hTb_sb[:, sl])
        nc.scalar.activation(u_full[:, sl], ps[:], EXP, bias=cas_sb[:],
                             scale=1.0)

    # ---- t/v: t[jc*128+p] via 1-wide bf16 matmuls on hTb ----
    v_col = cst.tile([P, NT], F32)
    t_ps = sps.tile([P, 512], F32, tag="sps")
    for r in range(NT):
        nc.tensor.matmul(t_ps[:, r:r + 1], hTb_sb[:, r * P:(r + 1) * P],
                         wd2b_sb)
        if r == 7:
            nc.scalar.activation(v_col[:, 0:8], t_ps[:, 0:8], EXP,
                                 bias=ba2_sb[:], scale=1.0)
    nc.scalar.activation(v_col[:, 8:NT], t_ps[:, 8:NT], EXP, bias=ba2_sb[:],
                         scale=1.0)

    # ---- hp chunks: hp0 = h @ W (f32), hp_aug = [hp0 | 1] bf16.
    # Algebra: P@hp0/rs + hp0 == P@hp/rs + hp - 2*b_proj; the constant
    # 2*b_proj row is added back on the HOST after the kernel returns. ----
    hp_aug = cst.tile([P, NT, 132], BF16)
    nc.vector.memset(hp_aug[:, :, 128:129], 1.0)

    def emit_hp(r):
        if r >= NT:
            return
        ps = sps.tile([P, 512], F32, tag="sps", name="hp_ps")
        nc.tensor.matmul(ps[:, :P], hTb_sb[:, r * P:(r + 1) * P], Wb_sb)
        nc.scalar.activation(hp_aug[:, r, 0:P], ps[:, :P], COPY)

    for r in range(NT):
        emit_hp(r)

    # ---- main psum: 16 accumulators [128, 129], packed 3 per bank ----
    mm_tiles = [mmp.tile([P, 512], F32, tag=f"mm{b}", name=f"mm{b}")
                for b in range(6)]

    def acc(ic):
        bank, slot = divmod(ic, 3)
        off = 130 * slot
        return mm_tiles[bank], off

    out_stage = cst.tile([P, NT, D], F32)

    # ---- main loop over j-chunks; z runs 3 chunks ahead of the product
    # so the pre-p0 DVE idle window does useful z work ----
    z_tiles = {}

    def emit_z(jc):
        if jc >= NT:
            return
        z_t = zp.tile([P, N], BF16, tag=f"z{jc % 3}", name="z_t")
        nc.vector.tensor_scalar(z_t[:], u_full[:], v_col[:, jc:jc + 1],
                                1.0, MULT, MAX)
        z_tiles[jc] = z_t

    for jc in range(3):
        emit_z(jc)
    for jc in range(NT):
        p_t = ppx.tile([P, N], BF16, tag=f"p{jc % 3}", name="p_t")
        z_t = z_tiles.pop(jc)
        if jc == NT - 1:
            # slabbed: chunk 15 alone gates the finalize, so letting its
            # matmuls chase the product 512 columns at a time pulls the
            # whole finalize cascade earlier
            for s4 in range(4):
                sl = slice(s4 * 512, (s4 + 1) * 512)
                nc.vector.tensor_tensor(p_t[:, sl], z_t[:, sl],
                                        aT_tiles[jc][:, sl], MULT)
        else:
            nc.vector.tensor_tensor(p_t[:], z_t[:], aT_tiles[jc][:], MULT)
        emit_z(jc + 3)
        for ic in range(NT):
            mt, off = acc(ic)
            # start=True clears PSUM has_written at BANK granularity on
            # TRN2: only the first accumulator of each bank may use it, or
            # it wipes its siblings' jc=0 contribution. The bank-wide clear
            # leaves the sibling slots "unwritten", so their jc=0 matmul
            # (start=False) overwrites rather than accumulates - correct on
            # every execution.
            nc.tensor.matmul(mt[:, off:off + 129],
                             p_t[:, ic * P:(ic + 1) * P],
                             hp_aug[:, jc, 0:129],
                             start=(jc == 0 and ic % 3 == 0),
                             stop=(jc == NT - 1),
                             skip_group_check=True)

    # ---- finalize in 4 groups of 4: out = psum/rowsum + hp0.
    # Two parallel pipelines per group: 2 chunks fused on DVE (stt), 2 on
    # ACT (scaled copy) + Pool (residual add), so no engine chain
    # serializes. ----
    out_r = out.rearrange("p (r d) -> p r d", r=NT)
    rinv_all = cst.tile([P, NT], F32)
    for g in range(4):
        ics = list(range(4 * g, 4 * g + 4))
        for ic in ics:
            mt, off = acc(ic)
            nc.vector.reciprocal(rinv_all[:, ic:ic + 1],
                                 mt[:, off + 128:off + 129])
        for ic in ics[2:]:
            mt, off = acc(ic)
            fin = rp.tile([P, D], F32, tag=f"fin{ic % 4}", name="fin")
            nc.scalar.activation(fin[:], mt[:, off:off + 128], COPY,
                                 scale=rinv_all[:, ic:ic + 1])
            nc.gpsimd.tensor_tensor(out_stage[:, ic, :], fin[:],
                                    hp_aug[:, ic, 0:P], ADD)
        for ic in ics[:2]:
            mt, off = acc(ic)
            nc.vector.scalar_tensor_tensor(out_stage[:, ic, :],
                                           mt[:, off:off + 128],
                                           rinv_all[:, ic:ic + 1],
                                           hp_aug[:, ic, 0:P], MULT, ADD)
        nc.sync.dma_start(out_r[:, 4 * g:4 * g + 4, :],
                          out_stage[:, 4 * g:4 * g + 4, :])


_CACHE = {}


def _get_compiled():
    if "nc" in _CACHE:
        return _CACHE["nc"], _CACHE["names"]

    nc = bacc.Bacc("TRN2", target_bir_lowering=False, debug=False)
    io = {}
    io["aT"] = nc.dram_tensor("aT", [N, N], BF16, kind="ExternalInput").ap()
    io["hx"] = nc.dram_tensor("hx", [D, N + 257], BF16, kind="ExternalInput").ap()

    io["cas"] = nc.dram_tensor("cas", [P, 1], F32, kind="ExternalInput").ap()
    io["ba2"] = nc.dram_tensor("ba2", [P, 1], F32, kind="ExternalInput").ap()
    io["out"] = nc.dram_tensor("out", [P, NT * D], F32, kind="ExternalOutput").ap()

    with tile.TileContext(nc) as tc:
        with ExitStack() as ctx:
            _build_kernel(ctx, tc, io)
    nc.compile()

    _CACHE["nc"] = nc
    _CACHE["names"] = list(io.keys())
    return nc, _CACHE["names"]


def _make_in_maps(a, h, W_proj, b_proj, w_att, b_att):
    import ml_dtypes
    bf16 = ml_dtypes.bfloat16

    a = np.asarray(a, dtype=np.float32)
    h = np.asarray(h, dtype=np.float32)
    W_proj = np.ascontiguousarray(W_proj, dtype=np.float32)
    b_proj = np.asarray(b_proj, dtype=np.float32).reshape(D)
    w_att = np.ascontiguousarray(w_att, dtype=np.float32)
    w_src, w_dst = w_att[:D], w_att[D:]

    ws2 = (W_proj @ w_src).astype(np.float32)
    ws2rb = np.tile(ws2[:, None], (1, P)).astype(bf16)
    wd2b = (W_proj @ w_dst).astype(np.float32).reshape(D, 1).astype(bf16)
    cas = np.full((P, 1), float(b_proj @ w_src), dtype=np.float32)
    ba2 = np.full((P, 1), float(b_proj @ w_dst) + float(b_att),
                  dtype=np.float32)

    in_maps = []
    for c in range(N_CORES):
        hT_c = np.ascontiguousarray(h[c].T)
        in_maps.append({
            "aT": np.ascontiguousarray(a[c].T).astype(bf16),
            "hx": np.ascontiguousarray(np.concatenate(
                [hT_c.astype(bf16), ws2rb, wd2b, W_proj.astype(bf16)],
                axis=1)),
            "cas": cas, "ba2": ba2,
        })
    return in_maps


def _get_executable():
    """Build (once) a sharded PJRT callable for the compiled Bass module.

    Mirrors concourse.bass2jax.run_bass_via_pjrt but keeps the jitted
    function so repeated calls don't retrace/recompile.
    """
    if "exe" in _CACHE:
        return _CACHE["exe"]

    import jax
    from jax.sharding import Mesh, PartitionSpec
    from jax.experimental.shard_map import shard_map
    from concourse import bass2jax, mybir as _mybir

    nc, _ = _get_compiled()
    bass2jax.install_neuronx_cc_hook()

    partition_name = (nc.partition_id_tensor.name
                      if nc.partition_id_tensor else None)
    in_names, out_names, out_avals, zero_outs = [], [], [], []
    for alloc in nc.m.functions[0].allocations:
        if not isinstance(alloc, _mybir.MemoryLocationSet):
            continue
        name = alloc.memorylocations[0].name
        if alloc.kind == "ExternalInput":
            if name != partition_name:
                in_names.append(name)
        elif alloc.kind == "ExternalOutput":
            shape = tuple(alloc.tensor_shape)
            dtype = _mybir.dt.np(alloc.dtype)
            out_names.append(name)
            out_avals.append(jax.core.ShapedArray(shape, dtype))
            zero_outs.append(np.zeros(shape, dtype))
    n_params = len(in_names)
    n_outs = len(out_avals)
    all_in_names = in_names + out_names + (
        [partition_name] if partition_name else [])
    donate = tuple(range(n_params, n_params + n_outs))

    def _body(*args):
        operands = list(args)
        if partition_name is not None:
            operands.append(bass2jax.partition_id_tensor())
        outs = bass2jax._bass_exec_p.bind(
            *operands,
            out_avals=tuple(out_avals),
            in_names=tuple(all_in_names),
            out_names=tuple(out_names),
            lowering_input_output_aliases=(),
            sim_require_finite=True,
            sim_require_nnan=True,
            nc=nc,
        )
        return tuple(outs)

    devices = jax.devices()[:N_CORES]
    mesh = Mesh(np.asarray(devices), ("core",))
    in_specs = (PartitionSpec("core"),) * (n_params + n_outs)
    out_specs = (PartitionSpec("core"),) * n_outs
    fn = jax.jit(
        shard_map(_body, mesh=mesh, in_specs=in_specs, out_specs=out_specs,
                  check_rep=False),
        donate_argnums=donate, keep_unused=True,
    )
    exe = {
        "fn": fn, "mesh": mesh, "in_names": in_names,
        "out_names": out_names, "out_avals": out_avals,
        "zero_outs": zero_outs, "n_params": n_params,
    }
    _CACHE["exe"] = exe
    return exe


def _concat_inputs(exe, in_maps):
    return [
        np.concatenate([np.asarray(in_maps[c][name])
                        for c in range(N_CORES)], axis=0)
        for name in exe["in_names"]
    ]


def _concat_zeros(exe):
    return [np.zeros((N_CORES * z.shape[0], *z.shape[1:]), z.dtype)
            for z in exe["zero_outs"]]


def kernel(a, h, W_proj, b_proj, w_att, b_att):
    exe = _get_executable()
    in_maps = _make_in_maps(a, h, W_proj, b_proj, w_att, b_att)
    out_arrs = exe["fn"](*_concat_inputs(exe, in_maps), *_concat_zeros(exe))
    i = exe["out_names"].index("out")
    raw = np.asarray(out_arrs[i]).reshape(N_CORES, P, NT, D)
    out = np.ascontiguousarray(raw.transpose(0, 2, 1, 3)).reshape(
        N_CORES, N, D)
    # the kernel computes P@hp0/rs + hp0 with hp0 = h@W (bias-free); the
    # missing constant row 2*b_proj is added here (exact algebra).
    out += 2.0 * np.asarray(b_proj, dtype=np.float32).reshape(1, 1, D)
    return out


if __name__ == "__main__":
    rng = np.random.default_rng(0)
    a = rng.random((B, N, N), dtype=np.float32)
    h = rng.standard_normal((B, N, D), dtype=np.float32)
    W_proj = (rng.standard_normal((D, D)) / np.sqrt(D)).astype(np.float32)
    b_proj = (rng.standard_normal(D) * 0.01).astype(np.float32)
    w_att = (rng.standard_normal(2 * D) / np.sqrt(2 * D)).astype(np.float32)
    b_att = np.float32(rng.standard_normal() * 0.01)

    got = kernel(a=a, h=h, W_proj=W_proj, b_proj=b_proj, w_att=w_att,
                 b_att=b_att)

    hp = h @ W_proj + b_proj
    s = hp @ w_att[:D]
    t = hp @ w_att[D:]
    e = np.maximum(s[:, :, None] + t[:, None, :] + b_att, 0.0)
    att = np.exp(e) * a
    att = att / att.sum(-1, keepdims=True)
    ref = att @ hp + hp

    err = np.abs(got - ref).max() / np.abs(ref).max()
    print("rel err:", err)
